# revision 61
# baseline (speedup 1.0000x reference)
"""Multi-head causal attention (B=4, T=2048, D=1024, H=16, HS=64) on 8 TRN2 cores.

Sharding: tensor-parallel over heads (2 heads/core) for QKV+attention, then an
AllToAll redistributes per-head context to token-parallel layout for the output
projection. The AllToAll is split into B=4 pieces (one per batch): each core
owns a 256-token slice of EVERY batch for the output projection, so piece b
can ship as soon as batch b's attention is done and its projection overlaps
batch b+1's attention.

Schedule: the attention inner loop is ACT-bound (exp ~984ns/ktile vs PE
~850ns/ktile), so the PE is fed from three sources: the score/AV matmuls
(AV software-pipelined one k-tile behind its exp), next-chunk phase-A
projection units pulled in as filler every other k-tile, and phase-C groups
of batch b-1 between chunks. A warmup AllToAll at t~0 absorbs the one-time
CC-ring setup (~60us on the first collective); memset-sourced warm matmuls
ramp the PE clock (0.65->2.4GHz takes 3us of continuous busy) at startup and
across the final a2a wait.

DMA: the queues are packet-rate-bound, so all large loads use long
per-partition lines -- x streams in half-batch tiles [128, 1024] (2KB lines),
wq/wk/wv are host-packed row-interleaved into one [128, 3072] DMA (6KB
lines), wp into [128, 8192] (16KB lines). Prefetch DMAs are emitted AFTER
the consumers of earlier data (per-queue completion semaphores are
cumulative in emission order: emitting them earlier creates false waits).

Per-phase notes:
  - qT/kT [ (h,e), t ] = W^T @ x^T with host-pretransposed weights/x.
  - v computed as [he, t] with Wv stationary (8 weight loads per chunk
    instead of 32), then PE-transposed (identity input) into the
    [keys, (h,e)] layout AV needs.
  - scoresT [k, q]: both heads' matmuls use disjoint PE row groups
    (tile_position auto-derived from base partitions 0/64) so they run
    concurrently in the array.
  - softmax without max-subtraction (scores ~ N(0,1); exp safe in fp32),
    1/sqrt(HS) folded into the ACT scale; causal masking multiplies a triu 0/1
    mask on only the 128-wide diagonal band.
  - AV uses an ones-augmented stationary operand [v_h|1] (M=65): output row 64
    accumulates the softmax denominator for free.
  - normalization: denominator reciprocal via the custom-DVE
    reciprocal_approx_fast (no ACT table swap -- an ACT Reciprocal forces a
    ~1.3us exp<->recip table reload twice per chunk), broadcast across
    partitions via a K=1 matmul with an ones row.
  - tail: the last batch's a2a is emitted before the deferred phC2 groups +
    warm spin so phase C of the final piece starts as soon as it lands.
"""
import numpy as np

import concourse.bass as bass
import concourse.tile as tile
from concourse import bacc, mybir
from concourse.bass_utils import run_bass_kernel_spmd

f32 = mybir.dt.float32
bf16 = mybir.dt.bfloat16

B, D, H, HS = 4, 1024, 16, 64
N_CORES = 8
HPC = H // N_CORES          # heads per core
QC = 512                    # q-chunk width
KT = 128                    # k-tile width
ND = D // 128               # din tiles

DT_NAME = "bf16"            # "bf16" | "f32"


def _np_dt(dt):
    import ml_dtypes
    return {f32: np.float32, bf16: ml_dtypes.bfloat16}[dt]


def build_nc(T=2048, dt_name=DT_NAME):
    DT = {"bf16": bf16, "f32": f32}[dt_name]
    BT = B * T
    SL = BT // N_CORES              # tokens per core in phase C
    NQC = T // QC                   # q-chunks per batch
    NTB = T // KT                   # k-tiles per batch
    HF = T // N_CORES               # tokens per core per a2a piece (=256)

    nc = bacc.Bacc("TRN2", target_bir_lowering=False, debug=False,
                   num_devices=N_CORES)

    xt_d = nc.dram_tensor("xt", [D, BT], DT, kind="ExternalInput").ap()
    # weights packed row-interleaved so each load is one DMA with long
    # per-partition lines (the DMA queues are packet-rate-bound)
    wqkv_d = nc.dram_tensor("wqkv", [128, 3 * ND * 128], DT,
                            kind="ExternalInput").ap()
    wp_d = nc.dram_tensor("wpp", [128, ND * D], DT,
                          kind="ExternalInput").ap()
    bp_d = nc.dram_tensor("bp", [D, 1], f32, kind="ExternalInput").ap()
    tri_d = nc.dram_tensor("triu", [128, 128], DT, kind="ExternalInput").ap()
    idn_d = nc.dram_tensor("ident", [128, 128], DT, kind="ExternalInput").ap()
    onesr_d = nc.dram_tensor("onesr", [65, 64], DT, kind="ExternalInput").ap()
    onesm_d = nc.dram_tensor("onesm", [128, NTB], DT,
                             kind="ExternalInput").ap()
    out_d = nc.dram_tensor("outT", [D, SL], DT, kind="ExternalOutput").ap()

    EXP = mybir.ActivationFunctionType.Exp

    with tile.TileContext(nc) as tc:
        with (
            tc.tile_pool(name="wts", bufs=1) as wts,
            tc.tile_pool(name="acts", bufs=1) as acts,
            tc.tile_pool(name="dram", bufs=1, space="DRAM") as dram,
        ):
            # a2a piece buffers: one per batch
            a2a_in = [dram.tile([N_CORES, 128, HF], DT, name=f"a2ai{b}")
                      for b in range(B)]
            a2a_out = [dram.tile([N_CORES, 128, HF], DT, name=f"a2ao{b}")
                       for b in range(B)]

            # per-batch activation tensors
            qT, kT, vA = [], [], []
            for b in range(B):
                qT.append(acts.tile([128, T], DT, name=f"qT{b}", tag=f"qT{b}"))
                kT.append(acts.tile([128, T], DT, name=f"kT{b}", tag=f"kT{b}"))
                vA.append(acts.tile([128, NTB * 130], DT, name=f"vA{b}",
                                    tag=f"vA{b}"))
            # persistent output accumulators: one [128, SL] tile per
            # 128-row output block, stored with a single 2KB-line DMA
            osA = [acts.tile([128, SL], DT, name=f"osA{m}", tag=f"osA{m}")
                   for m in range(ND)]

            wp_sb, bp_sb = [], []
            cxs = {}

            with (
                tc.tile_pool(name="pA", bufs=2) as pA,
                tc.tile_pool(name="pB", bufs=2) as pB,
                tc.tile_pool(name="pC", bufs=2) as pC,
                tc.tile_pool(name="psP", bufs=2, space="PSUM") as psP,
                tc.tile_pool(name="psS", bufs=2, space="PSUM") as psS,
                tc.tile_pool(name="psAV", bufs=1, space="PSUM") as psAV,
            ):
                # ---- PE warm-up on a memset tile: no DMA dependency, so
                # the PE clock starts ramping at t~0 instead of waiting
                # for the first weight DMAs ----
                warm_sb = wts.tile([128, 128], DT, name="warmsb",
                                   tag="warmsb")
                nc.vector.memset(warm_sb[:], 0.25)
                # ---- warm-up AllToAll: absorbs the one-time CC-ring
                # setup / cross-core skew (~60us on the first collective)
                # while phase A/B of batch 0 runs ----
                wa_in = dram.tile([N_CORES, 128, 8], DT, name="wa_i")
                wa_out = dram.tile([N_CORES, 128, 8], DT, name="wa_o")
                for c in range(N_CORES):
                    nc.sync.dma_start(wa_in[c], warm_sb[0:128, 0:8])
                nc.gpsimd.collective_compute(
                    "AllToAll", mybir.AluOpType.bypass,
                    replica_groups=[list(range(N_CORES))],
                    ins=[wa_in.opt()], outs=[wa_out.opt()])
                # ---- persistent loads (emission order = DMA priority:
                # the packed qkv weights first in ONE dma (6KB lines),
                # then the first x half-batch, then the rest) ----
                wqkv_sb = wts.tile([128, 3 * ND * 128], DT, name="wqkv",
                                   tag="wqkv")
                nc.sync.dma_start(wqkv_sb[:], wqkv_d[:])
                wq_sb = [wqkv_sb[:, j * 128:(j + 1) * 128]
                         for j in range(ND)]
                wk_sb = [wqkv_sb[:, (ND + j) * 128:(ND + j + 1) * 128]
                         for j in range(ND)]
                wv_sb = [wqkv_sb[:, (2 * ND + j) * 128:(2 * ND + j + 1) * 128]
                         for j in range(ND)]
                warm = psS.tile([128, 2 * QC], f32, name="warm",
                                tag="scb", bufs=2)
                for _ in range(32):
                    nc.tensor.matmul(warm[0:128, 0:128], warm_sb[:],
                                     warm_sb[:], start=True, stop=True)

                HT = T // 2     # tokens per x half-batch tile (2KB lines)

                def x_dmas(h, split=False):
                    i0 = (h // 2) * T + (h % 2) * HT
                    xt_t = []
                    for j in range(ND):
                        t = pA.tile([128, HT], DT, name=f"x{j}", tag=f"x{j}",
                                    bufs=3)
                        if split:
                            # startup only: land the first chunk's 512
                            # tokens first so phase A starts ~7us earlier
                            # (consumers dep on the sub-AP, not the tile)
                            nc.sync.dma_start(
                                t[:, 0:QC],
                                xt_d[j * 128:(j + 1) * 128, i0:i0 + QC])
                        else:
                            nc.sync.dma_start(
                                t[:], xt_d[j * 128:(j + 1) * 128,
                                           i0:i0 + HT])
                        xt_t.append(t)
                    if split:
                        for j, t in enumerate(xt_t):
                            nc.sync.dma_start(
                                t[:, QC:HT],
                                xt_d[j * 128:(j + 1) * 128,
                                     i0 + QC:i0 + HT])
                    return xt_t

                def x_dmas_c(ch):
                    # batch-0 chunks 0/1 as dedicated chunk-granularity
                    # tiles: DMA write deps are per-tile, so only separate
                    # tiles let chunk 0's projections start before the
                    # whole first half-batch lands (the x{j} pool's
                    # 3-buffer pipeline stays untouched)
                    xt_t = []
                    for j in range(ND):
                        t = pA.tile([128, QC], DT, name=f"xc{j}",
                                    tag=f"xc{ch}{j}", bufs=1)
                        nc.sync.dma_start(
                            t[:], xt_d[j * 128:(j + 1) * 128,
                                       ch * QC:(ch + 1) * QC])
                        xt_t.append(t)
                    return xt_t

                def phase_a_units(b, ch, pre_x=None, xoff=None):
                    """Next chunk's projections as ~850ns PE units, pulled
                    into the ACT-bound attention loop as PE filler."""
                    if ch == 0:
                        # ones columns of the augmented-V slots [v0|1|v1|1]
                        v3 = vA[b][:].rearrange("p (t c) -> p t c", c=130)
                        nc.vector.tensor_copy(v3[:, :, 64], onesm_sb[:])
                        nc.vector.tensor_copy(v3[:, :, 129], onesm_sb[:])
                    xt_t = pre_x
                    if xoff is None:
                        xoff = ch % 2
                    xsl = slice(xoff * QC, xoff * QC + QC)
                    sl = slice(ch * QC, (ch + 1) * QC)
                    st = {}

                    def mk_proj(key, w_sb, f32_out=True):
                        def u0():
                            st[key] = psP.tile([128, QC], f32, name=key,
                                               tag="proj", bufs=2)
                            for j in range(ND // 2):
                                nc.tensor.matmul(st[key][:], w_sb[j],
                                                 xt_t[j][:, xsl],
                                                 start=(j == 0), stop=False)
                        return u0

                    def mk_proj_fin(key, w_sb, done):
                        def u1():
                            for j in range(ND // 2, ND):
                                nc.tensor.matmul(st[key][:], w_sb[j],
                                                 xt_t[j][:, xsl],
                                                 start=False,
                                                 stop=(j == ND - 1))
                            done(st[key])
                        return u1

                    def q_done(pp):
                        nc.vector.tensor_copy(qT[b][:, sl], pp[:])

                    def k_done(kp):
                        nc.vector.tensor_copy(kT[b][:, sl], kp[:])

                    def v_done(vh):
                        # v computed as [he2, tok] with Wv stationary (8
                        # weight loads instead of 32); staged to SBUF for
                        # the PE transpose into AV's [keys, (h,e)] layout.
                        st['vhs'] = pB.tile([128, QC], DT, name="vhs",
                                            tag="vhs", bufs=2)
                        nc.vector.tensor_copy(st['vhs'][:], vh[:])

                    def u_trans():
                        vt = psP.tile([128, QC], DT, name="vt", tag="proj",
                                      bufs=2)
                        vhs = st['vhs']
                        for ts in range(QC // 128):
                            tsl = slice(ts * 128, (ts + 1) * 128)
                            nc.tensor.transpose(vt[:, tsl], vhs[:, tsl],
                                                ident_sb[:])
                        vt3 = vt[:].rearrange("p (ts c) -> p ts c", c=128)
                        va3 = vA[b][:].rearrange(
                            "p (t c) -> p t c",
                            c=130)[:, ch * 4:(ch + 1) * 4, :]
                        for h in range(HPC):
                            nc.vector.tensor_copy(
                                va3[:, :, h * 65:h * 65 + 64],
                                vt3[:, :, h * 64:(h + 1) * 64])

                    return [mk_proj('pp', wq_sb),
                            mk_proj_fin('pp', wq_sb, q_done),
                            mk_proj('vh', wv_sb),
                            mk_proj_fin('vh', wv_sb, v_done),
                            mk_proj('kp', wk_sb),
                            mk_proj_fin('kp', wk_sb, k_done),
                            u_trans]

                def attn_chunk(b, qc, pending, filler):
                    nj = 4 * qc + 4
                    av = [psAV.tile([65, QC], f32, name=f"av{h}",
                                    tag=f"av{h}", bufs=1)
                          for h in range(HPC)]

                    def emit_av(e, w, off, j, av=av, b=b, nj=nj):
                        for h in range(HPC):
                            lhs = vA[b][:, j * 130 + h * 65:
                                        j * 130 + h * 65 + 65]
                            nc.tensor.matmul(av[h][:, off:QC], lhs,
                                             e[:, h * w:(h + 1) * w],
                                             start=(j == 0),
                                             stop=(j == nj - 1))

                    if qc == 0 and b > 0 and filler:
                        # batch-opening chunk: give the scalar engine one
                        # unit's worth of time to drain the previous
                        # batch-end exp backlog before sc(0) needs its
                        # PSUM buffer back (sc WAR on exp)
                        with nc.named_scope("phAf"):
                            filler.popleft()()
                    prev = None
                    for j in range(nj):
                        jr = j - 4 * qc
                        off = max(jr, 0) * 128
                        w = QC - off
                        qsl = slice(qc * QC + off, (qc + 1) * QC)
                        # both heads' scores: disjoint PE row groups (base
                        # partitions 0/64) -> concurrent in the array
                        sc = psS.tile([128, 2 * QC], f32, name="scb",
                                      tag="scb", bufs=2)
                        for h in range(HPC):
                            hp = slice(h * 64, (h + 1) * 64)
                            nc.tensor.matmul(
                                sc[:, h * QC:h * QC + w],
                                kT[b][hp, j * 128:(j + 1) * 128],
                                qT[b][hp, qsl], start=True, stop=True)
                        # one exp for both heads via a strided AP
                        e = pB.tile([128, 2 * w], DT, name="exb",
                                    tag="exb", bufs=4)
                        sc3 = sc[:].rearrange("p (two q) -> p two q",
                                              two=2)[:, :, 0:w]
                        e3 = e[:].rearrange("p (two q) -> p two q", two=2)
                        nc.scalar.activation(e3, sc3, EXP,
                                             scale=1.0 / np.sqrt(HS))
                        if jr >= 0:
                            for h in range(HPC):
                                nc.vector.tensor_mul(
                                    e[:, h * w:h * w + 128],
                                    e[:, h * w:h * w + 128], triu_sb[:])
                        if j == 2 and pending:
                            # previous chunk's normalization matmuls land a
                            # couple of slots into this chunk so the PE FIFO
                            # never waits on the DVE reciprocal chain
                            for fn in pending:
                                fn()
                            pending.clear()
                        # software pipeline: AV(j-1) runs while the scalar
                        # engine computes exp(j), so the PE never waits on
                        # the exp+mask chain
                        if prev is not None:
                            emit_av(*prev)
                        prev = (e, w, off, j)
                        # the attention steady state is ACT-bound (exp ~984ns
                        # vs PE ~850ns per k-tile): feed the PE a next-chunk
                        # projection unit every other k-tile
                        if j % 2 == 1 and filler:
                            with nc.named_scope("phAf"):
                                filler.popleft()()
                    if filler:
                        with nc.named_scope("phAf"):
                            filler.popleft()()
                    emit_av(*prev)
                    # stage normalization: PSUM evacuation + reciprocal now
                    # (frees the av slots); the bcast matmul + ctx scaling
                    # are deferred into the next chunk via `pending`
                    from concourse.dve_ops import (
                        RECIP_APPROX_FAST_CONSTS as _RC,
                        RECIPROCAL_APPROX_FAST as _RF,
                    )
                    avs_l, rec_l = [], []
                    for h in range(HPC):
                        avs = pB.tile([65, QC], f32, name=f"avs{h}",
                                      tag=f"avs{h}", bufs=2)
                        nc.vector.tensor_copy(avs[:], av[h][:])
                        # full-tile reciprocal: single-partition [1,N] APs
                        # mis-execute the custom op; rows 0-63 are unused.
                        # bf16 out feeds the bcast matmul directly.
                        rec = pB.tile([65, QC], DT, name=f"rec{h}",
                                      tag=f"rec{h}", bufs=2)
                        nc.vector._custom_dve(
                            _RF, out=rec[:], in0=avs[:], s0=_RC["s0"],
                            s1=_RC["s1"], imm2=_RC["imm2"])
                        avs_l.append(avs)
                        rec_l.append(rec)

                    def finish_norm(b=b, qc=qc, avs_l=avs_l, rec_l=rec_l):
                        for h in range(HPC):
                            bcp = psP.tile([128, QC], f32, name="bcpp",
                                           tag="proj", bufs=2)
                            nc.tensor.matmul(
                                bcp[0:64, :], onesr_sb[64:65, :],
                                rec_l[h][64:65, :], start=True, stop=True,
                                tile_position=(64, 0))
                            ctx = pB.tile([64, QC], DT, name=f"ctx{h}",
                                          tag=f"ctx{h}", bufs=2)
                            nc.vector.tensor_mul(ctx[:], avs_l[h][0:64, :],
                                                 bcp[0:64, :])
                            nc.sync.dma_start(
                                a2a_in[b][2 * qc, h * 64:(h + 1) * 64, :],
                                ctx[:, 0:HF])
                            nc.sync.dma_start(
                                a2a_in[b][2 * qc + 1,
                                          h * 64:(h + 1) * 64, :],
                                ctx[:, HF:QC])

                    pending.append(finish_norm)

                def phc_load(b):
                    cxs[b] = []
                    for j in range(ND):
                        t = pC.tile([128, HF], DT, name=f"cx{j}",
                                    tag=f"cx{j}", bufs=2)
                        nc.sync.dma_start(t[:], a2a_out[b][j])
                        cxs[b].append(t)

                def phc_group(b, m):
                    op = psS.tile([128, 2 * QC], f32, name="op", tag="scb",
                                  bufs=2)
                    for j in range(ND):
                        nc.tensor.matmul(
                            op[0:128, 0:HF],
                            wp_all[:, j * D + m * 128:j * D + (m + 1) * 128],
                            cxs[b][j][:], start=(j == 0), stop=(j == ND - 1))
                    # accumulate into a persistent SBUF tile; the DMA out
                    # happens once per m-block with 2KB lines (4x fewer
                    # packets than per-batch 512B-line stores)
                    nc.vector.tensor_scalar_add(
                        osA[m][:, b * HF:(b + 1) * HF], op[0:128, 0:HF],
                        bp_sb[m][:])

                def do_a2a(b):
                    nc.gpsimd.collective_compute(
                        "AllToAll", mybir.AluOpType.bypass,
                        replica_groups=[list(range(N_CORES))],
                        ins=[a2a_in[b].opt()], outs=[a2a_out[b].opt()],
                        unique_tensors="Yes")

                x0 = x_dmas_c(0)
                triu_sb = wts.tile([128, 128], DT, name="triu", tag="triu")
                nc.sync.dma_start(triu_sb[:], tri_d[:])
                ident_sb = wts.tile([128, 128], DT, name="ident",
                                    tag="ident")
                nc.sync.dma_start(ident_sb[:], idn_d[:])
                onesr_sb = wts.tile([65, 64], DT, name="onesr", tag="onesr")
                nc.sync.dma_start(onesr_sb[:], onesr_d[:])
                onesm_sb = wts.tile([128, NTB], DT, name="onesm", tag="onesm")
                nc.sync.dma_start(onesm_sb[:], onesm_d[:])

                # Uniform (b, qc) pipeline: attn(b, qc) needs only phase-A
                # chunks 0..qc of batch b (causal), so each attention chunk
                # is followed by the NEXT phase-A chunk in global order --
                # batch 0's attention overlaps the fill, later batches behave
                # as before.  x DMAs prefetch two chunks ahead.  Phase-C of
                # piece b-1 is emitted only from qc==2 of batch b so the PE's
                # strict FIFO never reaches those matmuls before the
                # (asynchronous) AllToAll has delivered their inputs.
                from collections import deque
                seq = [(b, qc) for b in range(B) for qc in range(NQC)]
                # x half-batch tiles keyed by global half index i//2;
                # batch-0 chunks 0/1 use dedicated chunk tiles (x0/x0b)
                x_half = {}
                with nc.named_scope("phA0"):
                    for u in phase_a_units(*seq[0], pre_x=x0, xoff=0):
                        u()
                x0b = x_dmas_c(1)
                x_half[1] = x_dmas(1)

                pending, filler = [], deque()
                for i, (b, qc) in enumerate(seq):
                    if i + 1 < len(seq):
                        nb = seq[i + 1]
                        with nc.named_scope(f"phA{nb[0]}"):
                            if i + 1 == 1:
                                units = phase_a_units(*nb, pre_x=x0b,
                                                      xoff=0)
                            else:
                                units = phase_a_units(
                                    *nb, pre_x=x_half[(i + 1) // 2])
                            filler.extend(units)
                    with nc.named_scope(f"phB{b}"):
                        attn_chunk(b, qc, pending, filler)
                    if i == 1:
                        # wp packed [128, ND*D] (16KB lines, one DMA),
                        # deferred off the startup burst; first use (1,2)
                        wp_all = wts.tile([128, ND * D], DT, name="wpp",
                                          tag="wpp")
                        nc.sync.dma_start(wp_all[:], wp_d[:])
                        for j in range(ND):
                            wp_sb.append(wp_all[:, j * D:(j + 1) * D])
                        for m in range(ND):
                            t = wts.tile([128, 1], f32, name=f"bp{m}",
                                         tag=f"bp{m}")
                            nc.sync.dma_start(
                                t[:], bp_d[m * 128:(m + 1) * 128, :])
                            bp_sb.append(t)
                    if filler:
                        with nc.named_scope(f"phA{seq[i+1][0]}"):
                            while filler:
                                filler.popleft()()
                    if b >= 1 and qc >= 2:
                        with nc.named_scope(f"phC{b-1}"):
                            if qc == 2:
                                phc_load(b - 1)
                            if b < B - 1:
                                # the last batch's phC(b-1) groups are all
                                # deferred into the final-a2a bridge: they
                                # are useful PE work there in either CC
                                # latency mode
                                for m in range(4):
                                    phc_group(b - 1, (qc - 2) * 4 + m)
                    if qc == NQC - 1:
                        # the batch's last chunk must normalize + stage
                        # before its a2a piece ships
                        with nc.named_scope(f"phB{b}"):
                            for fn in pending:
                                fn()
                            pending.clear()
                        do_a2a(b)
                        if b == B - 1:
                            # bridge the final a2a wait with useful PE work
                            # (the deferred phC2 groups) plus a short warm
                            # spin so phase C of the last piece runs at speed
                            with nc.named_scope(f"phC{B-2}"):
                                for m in range(ND):
                                    phc_group(B - 2, m)
                            warm2 = psS.tile([128, 2 * QC], f32,
                                             name="warm2", tag="scb", bufs=2)
                            for _ in range(150):
                                nc.tensor.matmul(
                                    warm2[0:128, 0:128], warm_sb[:],
                                    warm_sb[:], start=True, stop=True)
                    nh = (i + 4) // 2
                    if nh not in x_half and nh < 2 * B:
                        # x prefetch emitted LAST: the per-queue DMA
                        # completion semaphore is cumulative in emission
                        # order, so emitting this before the (i+1) units
                        # would make them falsely wait on these tiles
                        with nc.named_scope(f"phA{nh//2}"):
                            x_half[nh] = x_dmas(nh)
                with nc.named_scope(f"phC{B-1}"):
                    phc_load(B - 1)
                    # j-outer with one accumulator region per 2KB PSUM
                    # bank (matmul `start` zeroes the whole bank): the
                    # first matmuls need only cx[0], so phase C pipelines
                    # with the cx DMA stream instead of waiting for all 8
                    for half in range(2):
                        ops = [psS.tile([128, 2 * QC], f32, name=f"op{k}",
                                        tag="scb", bufs=2)
                               for k in range(2)]
                        # region mi -> (buffer mi//2, bank-aligned column)
                        regs = [ops[mi // 2][:, (mi % 2) * QC:
                                             (mi % 2) * QC + HF]
                                for mi in range(4)]
                        for j in range(ND):
                            for mi in range(4):
                                m = half * 4 + mi
                                nc.tensor.matmul(
                                    regs[mi],
                                    wp_all[:, j * D + m * 128:
                                           j * D + (m + 1) * 128],
                                    cxs[B - 1][j][:],
                                    start=(j == 0), stop=(j == ND - 1))
                        for mi in range(4):
                            m = half * 4 + mi
                            nc.vector.tensor_scalar_add(
                                osA[m][:, (B - 1) * HF:B * HF],
                                regs[mi], bp_sb[m][:])
                            # alternate final stores across both HWDGE
                            # queues (SP + the idle ACT engine) so the
                            # last 4 stores drain in parallel
                            eng = nc.sync if mi % 2 == 0 else nc.scalar
                            eng.dma_start(
                                out_d[m * 128:(m + 1) * 128, :],
                                osA[m][:])

    nc.compile()
    return nc


def prep_inputs(x, Wq, Wk, Wv, Wp, bp, T, dt_name=DT_NAME):
    """Host-side sharding/layout prep. Returns in_maps for the 8 cores."""
    DT = {"bf16": bf16, "f32": f32}[dt_name]
    ndt = _np_dt(DT)
    BT = B * T
    NTB = T // KT

    x = np.asarray(x, np.float32)
    Wq = np.asarray(Wq, np.float32)
    Wk = np.asarray(Wk, np.float32)
    Wv = np.asarray(Wv, np.float32)
    Wp = np.asarray(Wp, np.float32)
    bp = np.asarray(bp, np.float32)

    ND = D // 128
    xt = np.ascontiguousarray(x.reshape(BT, D).T).astype(ndt)
    wp = np.ascontiguousarray(Wp.T).astype(ndt)
    # packed [128, ND*D]: row p = concat over j of Wp.T[128j+p, :]
    wpp = np.ascontiguousarray(
        wp.reshape(ND, 128, D).transpose(1, 0, 2).reshape(128, ND * D))
    bpc = np.ascontiguousarray(bp.reshape(D, 1))
    triu = np.triu(np.ones((128, 128), np.float32)).astype(ndt)
    ident = np.eye(128, dtype=np.float32).astype(ndt)
    onesr = np.ones((65, 64), np.float32).astype(ndt)
    onesm = np.ones((128, NTB), np.float32).astype(ndt)

    def wslice(W, c):
        # [H, D, HS] heads 2c,2c+1 -> [D, 128] as [d, (h_local, e)]
        return np.ascontiguousarray(
            W[2 * c:2 * c + 2].transpose(1, 0, 2).reshape(D, 2 * HS)
        ).astype(ndt)

    in_maps = []
    for c in range(N_CORES):
        # packed [128, 3*ND*128]: row p = concat over (w, j) of tile rows
        wqkv = np.concatenate(
            [wslice(W, c).reshape(ND, 128, 128) for W in (Wq, Wk, Wv)],
            axis=0).transpose(1, 0, 2).reshape(128, 3 * ND * 128)
        in_maps.append({
            "xt": xt, "wqkv": np.ascontiguousarray(wqkv), "wpp": wpp,
            "bp": bpc,
            "triu": triu, "ident": ident, "onesr": onesr, "onesm": onesm,
        })
    return in_maps


_NC_CACHE = {}


def kernel(x, Wq, Wk, Wv, Wp, bp):
    T = np.asarray(x).shape[1]
    key = (T, DT_NAME)
    if key not in _NC_CACHE:
        _NC_CACHE[key] = build_nc(T, DT_NAME)
    nc = _NC_CACHE[key]
    in_maps = prep_inputs(x, Wq, Wk, Wv, Wp, bp, T, DT_NAME)
    res = run_bass_kernel_spmd(nc, in_maps, list(range(N_CORES)))
    HF = T // N_CORES
    # core d, col c (c = b*HF + i)  <->  global token b*T + d*HF + i
    per_core = np.stack([res.results[c]["outT"].T for c in range(N_CORES)])
    per_core = per_core.reshape(N_CORES, B, HF, D).transpose(1, 0, 2, 3)
    return np.ascontiguousarray(
        per_core.reshape(B, T, D).astype(np.float32))



# revision 62
# speedup vs baseline: 1.1964x; 1.1964x over previous
"""Multi-head causal attention (B=4, T=2048, D=1024, H=16, HS=64) on 8 TRN2 cores.

Sharding: tensor-parallel over heads (2 heads/core) for QKV+attention, then an
AllToAll redistributes per-head context to token-parallel layout for the output
projection. The AllToAll is split into B=4 pieces (one per batch): each core
owns a 256-token slice of EVERY batch for the output projection, so piece b
can ship as soon as batch b's attention is done and its projection overlaps
batch b+1's attention.

Schedule: the attention inner loop is ACT-bound (exp ~984ns/ktile vs PE
~850ns/ktile), so the PE is fed from three sources: the score/AV matmuls
(AV software-pipelined one k-tile behind its exp), next-chunk phase-A
projection units pulled in as filler every other k-tile, and phase-C groups
of batch b-1 between chunks. A warmup AllToAll at t~0 absorbs the one-time
CC-ring setup (~60us on the first collective); memset-sourced warm matmuls
ramp the PE clock (0.65->2.4GHz takes 3us of continuous busy) at startup and
across the final a2a wait.

DMA: the queues are packet-rate-bound, so all large loads use long
per-partition lines -- x streams in half-batch tiles [128, 1024] (2KB lines),
wq/wk/wv are host-packed row-interleaved into one [128, 3072] DMA (6KB
lines), wp into [128, 8192] (16KB lines). Prefetch DMAs are emitted AFTER
the consumers of earlier data (per-queue completion semaphores are
cumulative in emission order: emitting them earlier creates false waits).

Per-phase notes:
  - qT/kT [ (h,e), t ] = W^T @ x^T with host-pretransposed weights/x.
  - v computed as [he, t] with Wv stationary (8 weight loads per chunk
    instead of 32), then PE-transposed (identity input) into the
    [keys, (h,e)] layout AV needs.
  - scoresT [k, q]: both heads' matmuls use disjoint PE row groups
    (tile_position auto-derived from base partitions 0/64) so they run
    concurrently in the array.
  - softmax without max-subtraction (scores ~ N(0,1); exp safe in fp32),
    1/sqrt(HS) folded into the ACT scale; causal masking multiplies a triu 0/1
    mask on only the 128-wide diagonal band.
  - AV uses an ones-augmented stationary operand [v_h|1] (M=65): output row 64
    accumulates the softmax denominator for free.
  - normalization: denominator reciprocal via the custom-DVE
    reciprocal_approx_fast (no ACT table swap -- an ACT Reciprocal forces a
    ~1.3us exp<->recip table reload twice per chunk), broadcast across
    partitions via a K=1 matmul with an ones row.
  - tail: the last batch's a2a is emitted before the deferred phC2 groups +
    warm spin so phase C of the final piece starts as soon as it lands.
"""
import numpy as np

import concourse.bass as bass
import concourse.tile as tile
from concourse import bacc, mybir
from concourse.bass_utils import run_bass_kernel_spmd

f32 = mybir.dt.float32
bf16 = mybir.dt.bfloat16

B, D, H, HS = 4, 1024, 16, 64
N_CORES = 8
HPC = H // N_CORES          # heads per core
QC = 512                    # q-chunk width
KT = 128                    # k-tile width
ND = D // 128               # din tiles

DT_NAME = "bf16"            # "bf16" | "f32"


def _np_dt(dt):
    import ml_dtypes
    return {f32: np.float32, bf16: ml_dtypes.bfloat16}[dt]


def build_nc(T=2048, dt_name=DT_NAME):
    DT = {"bf16": bf16, "f32": f32}[dt_name]
    BT = B * T
    SL = BT // N_CORES              # tokens per core in phase C
    NQC = T // QC                   # q-chunks per batch
    NTB = T // KT                   # k-tiles per batch
    HF = T // N_CORES               # tokens per core per a2a piece (=256)

    nc = bacc.Bacc("TRN2", target_bir_lowering=False, debug=False,
                   num_devices=N_CORES)

    xt_d = nc.dram_tensor("xt", [D, BT], DT, kind="ExternalInput").ap()
    # weights packed row-interleaved so each load is one DMA with long
    # per-partition lines (the DMA queues are packet-rate-bound)
    wqkv_d = nc.dram_tensor("wqkv", [128, 3 * ND * 128], DT,
                            kind="ExternalInput").ap()
    wp_d = nc.dram_tensor("wpp", [128, ND * D], DT,
                          kind="ExternalInput").ap()
    bp_d = nc.dram_tensor("bp", [D, 1], f32, kind="ExternalInput").ap()
    tri_d = nc.dram_tensor("triu", [128, 128], DT, kind="ExternalInput").ap()
    idn_d = nc.dram_tensor("ident", [128, 128], DT, kind="ExternalInput").ap()
    onesr_d = nc.dram_tensor("onesr", [65, 64], DT, kind="ExternalInput").ap()
    onesm_d = nc.dram_tensor("onesm", [128, NTB], DT,
                             kind="ExternalInput").ap()
    out_d = nc.dram_tensor("outT", [D, SL], DT, kind="ExternalOutput").ap()

    EXP = mybir.ActivationFunctionType.Exp

    with tile.TileContext(nc) as tc:
        with (
            tc.tile_pool(name="wts", bufs=1) as wts,
            tc.tile_pool(name="acts", bufs=1) as acts,
            tc.tile_pool(name="dram", bufs=1, space="DRAM") as dram,
        ):
            # a2a piece buffers: one per batch
            a2a_in = [dram.tile([N_CORES, 128, HF], DT, name=f"a2ai{b}")
                      for b in range(B)]
            a2a_out = [dram.tile([N_CORES, 128, HF], DT, name=f"a2ao{b}")
                       for b in range(B)]

            # per-batch activation tensors
            qT, kT, vA = [], [], []
            for b in range(B):
                qT.append(acts.tile([128, T], DT, name=f"qT{b}", tag=f"qT{b}"))
                kT.append(acts.tile([128, T], DT, name=f"kT{b}", tag=f"kT{b}"))
                vA.append(acts.tile([128, NTB * 130], DT, name=f"vA{b}",
                                    tag=f"vA{b}"))
            # persistent output accumulators: one [128, SL] tile per
            # 128-row output block, stored with a single 2KB-line DMA
            osA = [acts.tile([128, SL], DT, name=f"osA{m}", tag=f"osA{m}")
                   for m in range(ND)]

            wp_sb, bp_sb = [], []
            cxs = {}

            with (
                tc.tile_pool(name="pA", bufs=2) as pA,
                tc.tile_pool(name="pB", bufs=2) as pB,
                tc.tile_pool(name="pC", bufs=2) as pC,
                tc.tile_pool(name="psP", bufs=2, space="PSUM") as psP,
                tc.tile_pool(name="psS", bufs=2, space="PSUM") as psS,
                tc.tile_pool(name="psAV", bufs=1, space="PSUM") as psAV,
            ):
                # ---- PE warm-up on a memset tile: no DMA dependency, so
                # the PE clock starts ramping at t~0 instead of waiting
                # for the first weight DMAs ----
                warm_sb = wts.tile([128, 128], DT, name="warmsb",
                                   tag="warmsb")
                nc.vector.memset(warm_sb[:], 0.25)
                # ---- warm-up AllToAll: absorbs the one-time CC-ring
                # setup / cross-core skew (~60us on the first collective)
                # while phase A/B of batch 0 runs ----
                wa_in = dram.tile([N_CORES, 128, 8], DT, name="wa_i")
                wa_out = dram.tile([N_CORES, 128, 8], DT, name="wa_o")
                for c in range(N_CORES):
                    nc.sync.dma_start(wa_in[c], warm_sb[0:128, 0:8])
                nc.gpsimd.collective_compute(
                    "AllToAll", mybir.AluOpType.bypass,
                    replica_groups=[list(range(N_CORES))],
                    ins=[wa_in.opt()], outs=[wa_out.opt()])
                # ---- persistent loads (emission order = DMA priority:
                # the packed qkv weights first in ONE dma (6KB lines),
                # then the first x half-batch, then the rest) ----
                wqkv_sb = wts.tile([128, 3 * ND * 128], DT, name="wqkv",
                                   tag="wqkv")
                nc.sync.dma_start(wqkv_sb[:], wqkv_d[:])
                wq_sb = [wqkv_sb[:, j * 128:(j + 1) * 128]
                         for j in range(ND)]
                wk_sb = [wqkv_sb[:, (ND + j) * 128:(ND + j + 1) * 128]
                         for j in range(ND)]
                wv_sb = [wqkv_sb[:, (2 * ND + j) * 128:(2 * ND + j + 1) * 128]
                         for j in range(ND)]
                warm = psS.tile([128, 2 * QC], f32, name="warm",
                                tag="scb", bufs=2)
                for _ in range(32):
                    nc.tensor.matmul(warm[0:128, 0:128], warm_sb[:],
                                     warm_sb[:], start=True, stop=True)

                HT = T // 2     # tokens per x half-batch tile (2KB lines)

                def x_dmas(h, split=False):
                    i0 = (h // 2) * T + (h % 2) * HT
                    xt_t = []
                    for j in range(ND):
                        t = pA.tile([128, HT], DT, name=f"x{j}", tag=f"x{j}",
                                    bufs=3)
                        if split:
                            # startup only: land the first chunk's 512
                            # tokens first so phase A starts ~7us earlier
                            # (consumers dep on the sub-AP, not the tile)
                            nc.sync.dma_start(
                                t[:, 0:QC],
                                xt_d[j * 128:(j + 1) * 128, i0:i0 + QC])
                        else:
                            nc.sync.dma_start(
                                t[:], xt_d[j * 128:(j + 1) * 128,
                                           i0:i0 + HT])
                        xt_t.append(t)
                    if split:
                        for j, t in enumerate(xt_t):
                            nc.sync.dma_start(
                                t[:, QC:HT],
                                xt_d[j * 128:(j + 1) * 128,
                                     i0 + QC:i0 + HT])
                    return xt_t

                def x_dmas_c(ch):
                    # batch-0 chunks 0/1 as dedicated chunk-granularity
                    # tiles: DMA write deps are per-tile, so only separate
                    # tiles let chunk 0's projections start before the
                    # whole first half-batch lands (the x{j} pool's
                    # 3-buffer pipeline stays untouched)
                    xt_t = []
                    for j in range(ND):
                        t = pA.tile([128, QC], DT, name=f"xc{j}",
                                    tag=f"xc{ch}{j}", bufs=1)
                        nc.sync.dma_start(
                            t[:], xt_d[j * 128:(j + 1) * 128,
                                       ch * QC:(ch + 1) * QC])
                        xt_t.append(t)
                    return xt_t

                def phase_a_units(b, ch, pre_x=None, xoff=None):
                    """Next chunk's projections as ~850ns PE units, pulled
                    into the ACT-bound attention loop as PE filler."""
                    if ch == 0:
                        # ones columns of the augmented-V slots [v0|1|v1|1]
                        v3 = vA[b][:].rearrange("p (t c) -> p t c", c=130)
                        nc.vector.tensor_copy(v3[:, :, 64], onesm_sb[:])
                        nc.vector.tensor_copy(v3[:, :, 129], onesm_sb[:])
                    xt_t = pre_x
                    if xoff is None:
                        xoff = ch % 2
                    xsl = slice(xoff * QC, xoff * QC + QC)
                    sl = slice(ch * QC, (ch + 1) * QC)
                    st = {}

                    def mk_proj(key, w_sb, f32_out=True):
                        def u0():
                            st[key] = psP.tile([128, QC], f32, name=key,
                                               tag="proj", bufs=2)
                            for j in range(ND // 2):
                                nc.tensor.matmul(st[key][:], w_sb[j],
                                                 xt_t[j][:, xsl],
                                                 start=(j == 0), stop=False)
                        return u0

                    def mk_proj_fin(key, w_sb, done):
                        def u1():
                            for j in range(ND // 2, ND):
                                nc.tensor.matmul(st[key][:], w_sb[j],
                                                 xt_t[j][:, xsl],
                                                 start=False,
                                                 stop=(j == ND - 1))
                            done(st[key])
                        return u1

                    def q_done(pp):
                        nc.vector.tensor_copy(qT[b][:, sl], pp[:])

                    def k_done(kp):
                        nc.vector.tensor_copy(kT[b][:, sl], kp[:])

                    def v_done(vh):
                        # v computed as [he2, tok] with Wv stationary (8
                        # weight loads instead of 32); staged to SBUF for
                        # the PE transpose into AV's [keys, (h,e)] layout.
                        st['vhs'] = pB.tile([128, QC], DT, name="vhs",
                                            tag="vhs", bufs=2)
                        nc.vector.tensor_copy(st['vhs'][:], vh[:])

                    def u_trans():
                        vt = psP.tile([128, QC], DT, name="vt", tag="proj",
                                      bufs=2)
                        vhs = st['vhs']
                        for ts in range(QC // 128):
                            tsl = slice(ts * 128, (ts + 1) * 128)
                            nc.tensor.transpose(vt[:, tsl], vhs[:, tsl],
                                                ident_sb[:])
                        vt3 = vt[:].rearrange("p (ts c) -> p ts c", c=128)
                        va3 = vA[b][:].rearrange(
                            "p (t c) -> p t c",
                            c=130)[:, ch * 4:(ch + 1) * 4, :]
                        for h in range(HPC):
                            nc.vector.tensor_copy(
                                va3[:, :, h * 65:h * 65 + 64],
                                vt3[:, :, h * 64:(h + 1) * 64])

                    return [mk_proj('pp', wq_sb),
                            mk_proj_fin('pp', wq_sb, q_done),
                            mk_proj('vh', wv_sb),
                            mk_proj_fin('vh', wv_sb, v_done),
                            mk_proj('kp', wk_sb),
                            mk_proj_fin('kp', wk_sb, k_done),
                            u_trans]

                def attn_chunk(b, qc, pending, filler):
                    nj = 4 * qc + 4
                    av = [psAV.tile([65, QC], f32, name=f"av{h}",
                                    tag=f"av{h}", bufs=1)
                          for h in range(HPC)]

                    def emit_av(e, w, off, j, av=av, b=b, nj=nj):
                        for h in range(HPC):
                            lhs = vA[b][:, j * 130 + h * 65:
                                        j * 130 + h * 65 + 65]
                            nc.tensor.matmul(av[h][:, off:QC], lhs,
                                             e[:, h * w:(h + 1) * w],
                                             start=(j == 0),
                                             stop=(j == nj - 1))

                    if qc == 0 and b > 0 and filler:
                        # batch-opening chunk: give the scalar engine one
                        # unit's worth of time to drain the previous
                        # batch-end exp backlog before sc(0) needs its
                        # PSUM buffer back (sc WAR on exp)
                        with nc.named_scope("phAf"):
                            filler.popleft()()
                    prev = None
                    for j in range(nj):
                        jr = j - 4 * qc
                        off = max(jr, 0) * 128
                        w = QC - off
                        qsl = slice(qc * QC + off, (qc + 1) * QC)
                        # both heads' scores: disjoint PE row groups (base
                        # partitions 0/64) -> concurrent in the array
                        sc = psS.tile([128, 2 * QC], f32, name="scb",
                                      tag="scb", bufs=2)
                        for h in range(HPC):
                            hp = slice(h * 64, (h + 1) * 64)
                            nc.tensor.matmul(
                                sc[:, h * QC:h * QC + w],
                                kT[b][hp, j * 128:(j + 1) * 128],
                                qT[b][hp, qsl], start=True, stop=True)
                        # one exp for both heads via a strided AP
                        e = pB.tile([128, 2 * w], DT, name="exb",
                                    tag="exb", bufs=4)
                        sc3 = sc[:].rearrange("p (two q) -> p two q",
                                              two=2)[:, :, 0:w]
                        e3 = e[:].rearrange("p (two q) -> p two q", two=2)
                        nc.scalar.activation(e3, sc3, EXP,
                                             scale=1.0 / np.sqrt(HS))
                        if jr >= 0:
                            for h in range(HPC):
                                nc.vector.tensor_mul(
                                    e[:, h * w:h * w + 128],
                                    e[:, h * w:h * w + 128], triu_sb[:])
                        if j == 2 and pending:
                            # previous chunk's normalization matmuls land a
                            # couple of slots into this chunk so the PE FIFO
                            # never waits on the DVE reciprocal chain
                            for fn in pending:
                                fn()
                            pending.clear()
                        # software pipeline: AV(j-1) runs while the scalar
                        # engine computes exp(j), so the PE never waits on
                        # the exp+mask chain
                        if prev is not None:
                            emit_av(*prev)
                        prev = (e, w, off, j)
                        # the attention steady state is ACT-bound (exp ~984ns
                        # vs PE ~850ns per k-tile): feed the PE a next-chunk
                        # projection unit every other k-tile
                        if j % 2 == 1 and filler:
                            with nc.named_scope("phAf"):
                                filler.popleft()()
                    if filler:
                        with nc.named_scope("phAf"):
                            filler.popleft()()
                    emit_av(*prev)
                    # stage normalization: PSUM evacuation + reciprocal now
                    # (frees the av slots); the bcast matmul + ctx scaling
                    # are deferred into the next chunk via `pending`
                    from concourse.dve_ops import (
                        RECIP_APPROX_FAST_CONSTS as _RC,
                        RECIPROCAL_APPROX_FAST as _RF,
                    )
                    avs_l, rec_l = [], []
                    for h in range(HPC):
                        avs = pB.tile([65, QC], f32, name=f"avs{h}",
                                      tag=f"avs{h}", bufs=2)
                        nc.vector.tensor_copy(avs[:], av[h][:])
                        # full-tile reciprocal: single-partition [1,N] APs
                        # mis-execute the custom op; rows 0-63 are unused.
                        # bf16 out feeds the bcast matmul directly.
                        rec = pB.tile([65, QC], DT, name=f"rec{h}",
                                      tag=f"rec{h}", bufs=2)
                        nc.vector._custom_dve(
                            _RF, out=rec[:], in0=avs[:], s0=_RC["s0"],
                            s1=_RC["s1"], imm2=_RC["imm2"])
                        avs_l.append(avs)
                        rec_l.append(rec)

                    def finish_norm(b=b, qc=qc, avs_l=avs_l, rec_l=rec_l):
                        for h in range(HPC):
                            bcp = psP.tile([128, QC], f32, name="bcpp",
                                           tag="proj", bufs=2)
                            nc.tensor.matmul(
                                bcp[0:64, :], onesr_sb[64:65, :],
                                rec_l[h][64:65, :], start=True, stop=True,
                                tile_position=(64, 0))
                            ctx = pB.tile([64, QC], DT, name=f"ctx{h}",
                                          tag=f"ctx{h}", bufs=2)
                            nc.vector.tensor_mul(ctx[:], avs_l[h][0:64, :],
                                                 bcp[0:64, :])
                            nc.sync.dma_start(
                                a2a_in[b][2 * qc, h * 64:(h + 1) * 64, :],
                                ctx[:, 0:HF])
                            nc.sync.dma_start(
                                a2a_in[b][2 * qc + 1,
                                          h * 64:(h + 1) * 64, :],
                                ctx[:, HF:QC])

                    pending.append(finish_norm)

                def phc_load(b):
                    cxs[b] = []
                    for j in range(ND):
                        t = pC.tile([128, HF], DT, name=f"cx{j}",
                                    tag=f"cx{j}", bufs=2)
                        nc.sync.dma_start(t[:], a2a_out[b][j])
                        cxs[b].append(t)

                def phc_group(b, m):
                    op = psS.tile([128, 2 * QC], f32, name="op", tag="scb",
                                  bufs=2)
                    for j in range(ND):
                        nc.tensor.matmul(
                            op[0:128, 0:HF],
                            wp_all[:, j * D + m * 128:j * D + (m + 1) * 128],
                            cxs[b][j][:], start=(j == 0), stop=(j == ND - 1))
                    # accumulate into a persistent SBUF tile; the DMA out
                    # happens once per m-block with 2KB lines (4x fewer
                    # packets than per-batch 512B-line stores)
                    nc.vector.tensor_scalar_add(
                        osA[m][:, b * HF:(b + 1) * HF], op[0:128, 0:HF],
                        bp_sb[m][:])

                def do_a2a(b):
                    nc.gpsimd.collective_compute(
                        "AllToAll", mybir.AluOpType.bypass,
                        replica_groups=[list(range(N_CORES))],
                        ins=[a2a_in[b].opt()], outs=[a2a_out[b].opt()],
                        unique_tensors="Yes")

                x0 = x_dmas_c(0)
                triu_sb = wts.tile([128, 128], DT, name="triu", tag="triu")
                nc.sync.dma_start(triu_sb[:], tri_d[:])
                ident_sb = wts.tile([128, 128], DT, name="ident",
                                    tag="ident")
                nc.sync.dma_start(ident_sb[:], idn_d[:])
                onesr_sb = wts.tile([65, 64], DT, name="onesr", tag="onesr")
                nc.sync.dma_start(onesr_sb[:], onesr_d[:])
                onesm_sb = wts.tile([128, NTB], DT, name="onesm", tag="onesm")
                nc.sync.dma_start(onesm_sb[:], onesm_d[:])

                # Uniform (b, qc) pipeline: attn(b, qc) needs only phase-A
                # chunks 0..qc of batch b (causal), so each attention chunk
                # is followed by the NEXT phase-A chunk in global order --
                # batch 0's attention overlaps the fill, later batches behave
                # as before.  x DMAs prefetch two chunks ahead.  Phase-C of
                # piece b-1 is emitted only from qc==2 of batch b so the PE's
                # strict FIFO never reaches those matmuls before the
                # (asynchronous) AllToAll has delivered their inputs.
                from collections import deque
                seq = [(b, qc) for b in range(B) for qc in range(NQC)]
                # x half-batch tiles keyed by global half index i//2;
                # batch-0 chunks 0/1 use dedicated chunk tiles (x0/x0b)
                x_half = {}
                with nc.named_scope("phA0"):
                    for u in phase_a_units(*seq[0], pre_x=x0, xoff=0):
                        u()
                x0b = x_dmas_c(1)
                x_half[1] = x_dmas(1)

                pending, filler = [], deque()
                for i, (b, qc) in enumerate(seq):
                    if i + 1 < len(seq):
                        nb = seq[i + 1]
                        with nc.named_scope(f"phA{nb[0]}"):
                            if i + 1 == 1:
                                units = phase_a_units(*nb, pre_x=x0b,
                                                      xoff=0)
                            else:
                                units = phase_a_units(
                                    *nb, pre_x=x_half[(i + 1) // 2])
                            filler.extend(units)
                    with nc.named_scope(f"phB{b}"):
                        attn_chunk(b, qc, pending, filler)
                    if i == 1:
                        # wp packed [128, ND*D] (16KB lines, one DMA),
                        # deferred off the startup burst; first use (1,2)
                        wp_all = wts.tile([128, ND * D], DT, name="wpp",
                                          tag="wpp")
                        nc.sync.dma_start(wp_all[:], wp_d[:])
                        for j in range(ND):
                            wp_sb.append(wp_all[:, j * D:(j + 1) * D])
                        for m in range(ND):
                            t = wts.tile([128, 1], f32, name=f"bp{m}",
                                         tag=f"bp{m}")
                            nc.sync.dma_start(
                                t[:], bp_d[m * 128:(m + 1) * 128, :])
                            bp_sb.append(t)
                    if filler:
                        with nc.named_scope(f"phA{seq[i+1][0]}"):
                            while filler:
                                filler.popleft()()
                    if b >= 1 and qc >= 2:
                        with nc.named_scope(f"phC{b-1}"):
                            if qc == 2:
                                phc_load(b - 1)
                            if b < B - 1:
                                # the last batch's phC(b-1) groups are all
                                # deferred into the final-a2a bridge: they
                                # are useful PE work there in either CC
                                # latency mode
                                for m in range(4):
                                    phc_group(b - 1, (qc - 2) * 4 + m)
                    if qc == NQC - 1:
                        # the batch's last chunk must normalize + stage
                        # before its a2a piece ships
                        with nc.named_scope(f"phB{b}"):
                            for fn in pending:
                                fn()
                            pending.clear()
                        do_a2a(b)
                        if b == B - 1:
                            # bridge the final a2a wait with useful PE work
                            # (the deferred phC2 groups) plus a short warm
                            # spin so phase C of the last piece runs at speed
                            with nc.named_scope(f"phC{B-2}"):
                                for m in range(ND):
                                    phc_group(B - 2, m)
                            warm2 = psS.tile([128, 2 * QC], f32,
                                             name="warm2", tag="scb", bufs=2)
                            for _ in range(120):
                                nc.tensor.matmul(
                                    warm2[0:128, 0:128], warm_sb[:],
                                    warm_sb[:], start=True, stop=True)
                    nh = (i + 4) // 2
                    if nh not in x_half and nh < 2 * B:
                        # x prefetch emitted LAST: the per-queue DMA
                        # completion semaphore is cumulative in emission
                        # order, so emitting this before the (i+1) units
                        # would make them falsely wait on these tiles
                        with nc.named_scope(f"phA{nh//2}"):
                            x_half[nh] = x_dmas(nh)
                with nc.named_scope(f"phC{B-1}"):
                    phc_load(B - 1)
                    # j-outer with one accumulator region per 2KB PSUM
                    # bank (matmul `start` zeroes the whole bank): the
                    # first matmuls need only cx[0], so phase C pipelines
                    # with the cx DMA stream instead of waiting for all 8
                    for half in range(2):
                        ops = [psS.tile([128, 2 * QC], f32, name=f"op{k}",
                                        tag="scb", bufs=2)
                               for k in range(2)]
                        # region mi -> (buffer mi//2, bank-aligned column)
                        regs = [ops[mi // 2][:, (mi % 2) * QC:
                                             (mi % 2) * QC + HF]
                                for mi in range(4)]
                        for j in range(ND):
                            for mi in range(4):
                                m = half * 4 + mi
                                nc.tensor.matmul(
                                    regs[mi],
                                    wp_all[:, j * D + m * 128:
                                           j * D + (m + 1) * 128],
                                    cxs[B - 1][j][:],
                                    start=(j == 0), stop=(j == ND - 1))
                        for mi in range(4):
                            m = half * 4 + mi
                            nc.vector.tensor_scalar_add(
                                osA[m][:, (B - 1) * HF:B * HF],
                                regs[mi], bp_sb[m][:])
                            nc.sync.dma_start(
                                out_d[m * 128:(m + 1) * 128, :],
                                osA[m][:])

    nc.compile()
    return nc


def prep_inputs(x, Wq, Wk, Wv, Wp, bp, T, dt_name=DT_NAME):
    """Host-side sharding/layout prep. Returns in_maps for the 8 cores."""
    DT = {"bf16": bf16, "f32": f32}[dt_name]
    ndt = _np_dt(DT)
    BT = B * T
    NTB = T // KT

    x = np.asarray(x, np.float32)
    Wq = np.asarray(Wq, np.float32)
    Wk = np.asarray(Wk, np.float32)
    Wv = np.asarray(Wv, np.float32)
    Wp = np.asarray(Wp, np.float32)
    bp = np.asarray(bp, np.float32)

    ND = D // 128
    xt = np.ascontiguousarray(x.reshape(BT, D).T).astype(ndt)
    wp = np.ascontiguousarray(Wp.T).astype(ndt)
    # packed [128, ND*D]: row p = concat over j of Wp.T[128j+p, :]
    wpp = np.ascontiguousarray(
        wp.reshape(ND, 128, D).transpose(1, 0, 2).reshape(128, ND * D))
    bpc = np.ascontiguousarray(bp.reshape(D, 1))
    triu = np.triu(np.ones((128, 128), np.float32)).astype(ndt)
    ident = np.eye(128, dtype=np.float32).astype(ndt)
    onesr = np.ones((65, 64), np.float32).astype(ndt)
    onesm = np.ones((128, NTB), np.float32).astype(ndt)

    def wslice(W, c):
        # [H, D, HS] heads 2c,2c+1 -> [D, 128] as [d, (h_local, e)]
        return np.ascontiguousarray(
            W[2 * c:2 * c + 2].transpose(1, 0, 2).reshape(D, 2 * HS)
        ).astype(ndt)

    in_maps = []
    for c in range(N_CORES):
        # packed [128, 3*ND*128]: row p = concat over (w, j) of tile rows
        wqkv = np.concatenate(
            [wslice(W, c).reshape(ND, 128, 128) for W in (Wq, Wk, Wv)],
            axis=0).transpose(1, 0, 2).reshape(128, 3 * ND * 128)
        in_maps.append({
            "xt": xt, "wqkv": np.ascontiguousarray(wqkv), "wpp": wpp,
            "bp": bpc,
            "triu": triu, "ident": ident, "onesr": onesr, "onesm": onesm,
        })
    return in_maps


_NC_CACHE = {}


def kernel(x, Wq, Wk, Wv, Wp, bp):
    T = np.asarray(x).shape[1]
    key = (T, DT_NAME)
    if key not in _NC_CACHE:
        _NC_CACHE[key] = build_nc(T, DT_NAME)
    nc = _NC_CACHE[key]
    in_maps = prep_inputs(x, Wq, Wk, Wv, Wp, bp, T, DT_NAME)
    res = run_bass_kernel_spmd(nc, in_maps, list(range(N_CORES)))
    HF = T // N_CORES
    # core d, col c (c = b*HF + i)  <->  global token b*T + d*HF + i
    per_core = np.stack([res.results[c]["outT"].T for c in range(N_CORES)])
    per_core = per_core.reshape(N_CORES, B, HF, D).transpose(1, 0, 2, 3)
    return np.ascontiguousarray(
        per_core.reshape(B, T, D).astype(np.float32))

